# revision 10
# baseline (speedup 1.0000x reference)
"""Biased MHSA Trainium2 kernel (8-core SPMD).

Sharding: core c -> (batch b = c//2, head-group g = c%2); each core computes
attention for 4 of the 8 heads of one batch and the partial output projection
for those heads. Host sums the two head-group partials per batch and adds bo.

Per-core device kernel (all in fp32, matmuls via float32r):
  - Q^T,K^T computed in [feature, token] layout; V in natural [token, feature]
    layout augmented with a ones column (gives softmax denominator for free).
  - S^T[k,q] = K_h^T.T @ (Q_h^T/8) per 128-key tile; DVE adds bias^T tile;
    ACT exp in place; U_aug^T[d|1, q] += V_aug.T @ P^T accumulated over keys.
  - Deferred softmax normalization: A^T = U^T * (1/r) with r broadcast across
    partitions via a 0-stride DMA; + bv.
  - O[tok, 512] = A^T.T @ wo accumulated over the 4 heads.
"""

import sys

if "/opt/trn_rl_repo" not in sys.path:
    sys.path.insert(0, "/opt/trn_rl_repo")

from contextlib import ExitStack

import numpy as np

import concourse.bass as bass
from concourse import bacc
import concourse.tile as tile
from concourse import mybir
from concourse.bass_utils import run_bass_kernel_spmd

B, N, D = 4, 2048, 512
H, DH = 8, 64
HG = 4  # heads per core
GD = HG * DH  # 256 features per core
P = 128
QQ = 512  # q processed in chunks of 512
NKT = N // P  # 16 key tiles
NQQ = N // QQ  # 4 q chunks
NTOK = N // P  # 16 token tiles
KC = D // P  # 4 contraction chunks for projections
F32 = mybir.dt.float32
F32R = mybir.dt.float32r


def build_program():
    nc = bacc.Bacc("TRN2", target_bir_lowering=False)
    xT = nc.dram_tensor("xT", [D, N], F32R, kind="ExternalInput")
    biasT = nc.dram_tensor("biasT", [N, N], F32, kind="ExternalInput")
    wq = nc.dram_tensor("wq", [D, GD], F32R, kind="ExternalInput")
    wk = nc.dram_tensor("wk", [D, GD], F32R, kind="ExternalInput")
    wv = nc.dram_tensor("wv", [D, GD], F32R, kind="ExternalInput")
    wo = nc.dram_tensor("wo", [GD, D], F32R, kind="ExternalInput")
    bq = nc.dram_tensor("bq", [GD], F32, kind="ExternalInput")
    bk = nc.dram_tensor("bk", [GD], F32, kind="ExternalInput")
    bv = nc.dram_tensor("bv", [GD], F32, kind="ExternalInput")
    out = nc.dram_tensor("out", [N, D], F32, kind="ExternalOutput")

    with tile.TileContext(nc) as tc, ExitStack() as ctx:
        const = ctx.enter_context(tc.tile_pool(name="const", bufs=1))
        big = ctx.enter_context(tc.tile_pool(name="big", bufs=1))
        bias_pool = ctx.enter_context(tc.tile_pool(name="biasp", bufs=3))
        p_pool = ctx.enter_context(tc.tile_pool(name="probs", bufs=4))
        small = ctx.enter_context(tc.tile_pool(name="small", bufs=4))
        o_pool = ctx.enter_context(tc.tile_pool(name="outp", bufs=3))
        psum_mm = ctx.enter_context(tc.tile_pool(name="psum_mm", bufs=3, space="PSUM"))
        psum_u = ctx.enter_context(tc.tile_pool(name="psum_u", bufs=1, space="PSUM"))

        # ---- load inputs ----
        xT_s = big.tile([P, KC, N], F32R)  # x^T as [128, kc, tok]
        nc.sync.dma_start(out=xT_s, in_=xT.rearrange("(kc p) n -> p kc n", p=P))
        wq_s = const.tile([P, KC, GD], F32R)
        nc.sync.dma_start(out=wq_s, in_=wq.rearrange("(kc p) f -> p kc f", p=P))
        wk_s = const.tile([P, KC, GD], F32R)
        nc.sync.dma_start(out=wk_s, in_=wk.rearrange("(kc p) f -> p kc f", p=P))
        wv_s = const.tile([P, KC, GD], F32R)
        nc.sync.dma_start(out=wv_s, in_=wv.rearrange("(kc p) f -> p kc f", p=P))
        wo_s = const.tile([DH, HG, D], F32R)  # per-head wo rows: [64, h, 512]
        nc.sync.dma_start(out=wo_s, in_=wo.rearrange("(h p) d -> p h d", p=DH))
        bq_s = const.tile([P, 2], F32)
        nc.sync.dma_start(out=bq_s, in_=bq.rearrange("(fc p) -> p fc", p=P))
        bk_s = const.tile([P, 2], F32)
        nc.sync.dma_start(out=bk_s, in_=bk.rearrange("(fc p) -> p fc", p=P))
        bv_s = const.tile([DH, HG], F32)
        nc.sync.dma_start(out=bv_s, in_=bv.rearrange("(h p) -> p h", p=DH))
        bqs = const.tile([P, 2], F32)  # bq * 0.125 (scale folded into Q)
        nc.vector.tensor_scalar_mul(bqs, bq_s, 0.125)

        # ---- projections ----
        # Q^T, K^T: [128, fc, tok] (feature on partitions; head h lives at
        # partitions (h%2)*64..+64 of chunk fc=h//2)
        qT = big.tile([P, 2, N], F32R)
        kT = big.tile([P, 2, N], F32R)
        for name, w_s, dst, b_ap, scale in (
            ("q", wq_s, qT, bqs, 0.125),
            ("k", wk_s, kT, bk_s, 1.0),
        ):
            for fc in range(2):
                for nn in range(NQQ):
                    ps = psum_mm.tile([P, QQ], F32, tag="mm")
                    for kc in range(KC):
                        nc.tensor.matmul(
                            ps,
                            (w_s[:, kc, fc * P : (fc + 1) * P]),
                            (xT_s[:, kc, nn * QQ : (nn + 1) * QQ]),
                            start=(kc == 0),
                            stop=(kc == KC - 1),
                        )
                    # (x@w + b) * scale  ==  psum*scale + b*scale
                    nc.vector.tensor_scalar(
                        dst[:, fc, nn * QQ : (nn + 1) * QQ],
                        ps,
                        scale,
                        b_ap[:, fc : fc + 1],
                        op0=mybir.AluOpType.mult,
                        op1=mybir.AluOpType.add,
                    )

        # V natural layout, augmented ones column: vaug[128tok, h, kt, 65]
        vaug = big.tile([P, HG, NKT, DH + 1], F32R)
        nc.vector.memset(vaug[:, :, :, DH : DH + 1].bitcast(F32), 1.0)
        for kt in range(NKT):
            ps = psum_mm.tile([P, GD], F32, tag="mm")
            for kc in range(KC):
                nc.tensor.matmul(
                    ps,
                    (xT_s[:, kc, kt * P : (kt + 1) * P]),
                    (wv_s[:, kc, :]),
                    start=(kc == 0),
                    stop=(kc == KC - 1),
                )
            # scatter heads into vaug[:, h, kt, 0:64] (bv folded in later)
            nc.vector.tensor_copy(
                vaug[:, :, kt, 0:DH],
                ps.rearrange("p (h d) -> p h d", h=HG),
            )

        # ---- attention ----
        aT = big.tile([DH, HG, N], F32R)  # A^T = normalized attn out, [64, h, tok]
        for qq in range(NQQ):
            qsl = slice(qq * QQ, (qq + 1) * QQ)
            pu = [
                psum_u.tile([DH + 1, QQ], F32, tag=f"u{h}", name=f"pu{h}")
                for h in range(HG)
            ]
            for kt in range(NKT):
                bt = bias_pool.tile([P, QQ], F32)
                nc.sync.dma_start(
                    out=bt, in_=biasT[kt * P : (kt + 1) * P, qsl]
                )
                for h in range(HG):
                    ho = (h % 2) * DH
                    fo = h // 2
                    ps = psum_mm.tile([P, QQ], F32, tag="mm")
                    nc.tensor.matmul(
                        ps,
                        (kT[ho : ho + DH, fo, kt * P : (kt + 1) * P]),
                        (qT[ho : ho + DH, fo, qsl]),
                        start=True,
                        stop=True,
                    )
                    sc = p_pool.tile([P, QQ], F32, tag="sc")
                    nc.vector.tensor_tensor(sc, ps, bt, op=mybir.AluOpType.add)
                    sp = p_pool.tile([P, QQ], F32R, tag="sp")
                    nc.scalar.activation(sp, sc, mybir.ActivationFunctionType.Exp)
                    nc.tensor.matmul(
                        pu[h],
                        (vaug[:, h, kt, :]),
                        (sp),
                        start=(kt == 0),
                        stop=(kt == NKT - 1),
                    )
            # normalize: A^T_h = U^T_h * (1/r) + bv_h
            for h in range(HG):
                rec = small.tile([DH + 1, QQ], F32, tag="rec")
                nc.vector.reciprocal(rec[DH : DH + 1, :], pu[h][DH : DH + 1, :])
                rec0 = small.tile([1, QQ], F32, tag="rec0")
                nc.sync.dma_start(out=rec0, in_=rec[DH : DH + 1, :])
                bc = small.tile([DH, QQ], F32, tag="bc")
                nc.gpsimd.partition_broadcast(bc, rec0[0:1, :])
                nc.vector.tensor_tensor(
                    aT[:, h, qsl], pu[h][0:DH, :], bc, op=mybir.AluOpType.mult
                )
                nc.vector.tensor_scalar_add(
                    aT[:, h, qsl], aT[:, h, qsl], bv_s[:, h : h + 1]
                )

        # ---- output projection: O[tok, 512] = sum_h A_h @ wo_h ----
        for t in range(NTOK):
            ps = psum_mm.tile([P, D], F32, tag="mm")
            for h in range(HG):
                nc.tensor.matmul(
                    ps,
                    (aT[:, h, t * P : (t + 1) * P]),
                    (wo_s[:, h, :]),
                    start=(h == 0),
                    stop=(h == HG - 1),
                )
            ob = o_pool.tile([P, D], F32)
            nc.scalar.copy(ob, ps)
            nc.sync.dma_start(out=out[t * P : (t + 1) * P, :], in_=ob)

    nc.compile()
    return nc


_NC = None


def _get_nc():
    global _NC
    if _NC is None:
        _NC = build_program()
    return _NC


def make_in_maps(x, attn_bias, wq, bq, wk, bk, wv, bv, wo, bo):
    x = np.asarray(x, np.float32)
    attn_bias = np.asarray(attn_bias, np.float32)
    in_maps = []
    for c in range(8):
        b, g = c // 2, c % 2
        sl = slice(g * GD, (g + 1) * GD)
        in_maps.append(
            {
                "xT": np.ascontiguousarray(x[b].T),
                "biasT": np.ascontiguousarray(attn_bias[b, 0].T),
                "wq": np.ascontiguousarray(np.asarray(wq, np.float32)[:, sl]),
                "wk": np.ascontiguousarray(np.asarray(wk, np.float32)[:, sl]),
                "wv": np.ascontiguousarray(np.asarray(wv, np.float32)[:, sl]),
                "wo": np.ascontiguousarray(np.asarray(wo, np.float32)[sl, :]),
                "bq": np.ascontiguousarray(np.asarray(bq, np.float32)[sl]),
                "bk": np.ascontiguousarray(np.asarray(bk, np.float32)[sl]),
                "bv": np.ascontiguousarray(np.asarray(bv, np.float32)[sl]),
            }
        )
    return in_maps


def gather_output(results, bo):
    bo = np.asarray(bo, np.float32)
    out = np.empty((B, N, D), np.float32)
    for b in range(B):
        out[b] = results[2 * b]["out"] + results[2 * b + 1]["out"] + bo[None, :]
    return out


def kernel(x, attn_bias, wq, bq, wk, bk, wv, bv, wo, bo, _trace=False):
    nc = _get_nc()
    in_maps = make_in_maps(x, attn_bias, wq, bq, wk, bk, wv, bv, wo, bo)
    res = run_bass_kernel_spmd(nc, in_maps, core_ids=list(range(8)), trace=_trace)
    out = gather_output(res.results, bo)
    if _trace:
        kernel.last_results = res
    return out
